# revision 24
# baseline (speedup 1.0000x reference)
"""Trainium2 Bass kernel for ConvTranspose3d(32->64, k=3, s=2, p=1) + inference
BatchNorm + per-(sample,channel) spatial mean subtraction.

Math: bias / beta / running_mean cancel exactly in the mean subtraction:
    out = A_c * (convT(x) - mean_spatial(convT(x))),  A_c = gamma/sqrt(var+eps)
A_c is folded into the weights on the host, so the device epilogue is a
single per-channel bias add.

Decomposition: stride-2 transpose conv -> 8 output parity classes.
x arrives pre-cast to bf16; the 4 (sh,sw) spatial shift variants live in 4
partition groups of one SBUF tensor T1 (128 = 4x32ci partitions), loaded
directly from a host-padded flat copy of x at 4 byte offsets (pad-free
32x32 plane layout; shift overruns read neighbouring-plane garbage that
only feeds trimmed grid positions).  Per (ph,pw) class both d-parities are
computed by 2 matmuls (M=128 = pd x co; pass 1 reads plane jd with kd=1|2
weights, pass 2 reads plane jd+1 with kd=0 weights in the upper half).

The spatial mean: per shift-group the needed 12 box sums of x equal plain
prefix-range sums of that group's own shifted copy (shifted data + fixed
local range == fixed data + shifted range), so they are computed directly
on T1's 128 partitions from per-plane full sums + last-row/col edge sums,
then fed through 12 tiny matmuls into the epilogue bias.  Epilogues for
jd0-4 run bias-free (plain copy) and are fixed up in stag afterwards, so
the PE never stalls on the mean during the load ramp.

Sharding: data-parallel, one sample per core (B=8, 8 cores).
"""

import numpy as np

B, CIN, COUT = 8, 32, 64
D, H, W = 16, 32, 32
DO, HO, WO = 31, 63, 63
EPS = 1e-5
NSPAT = DO * HO * WO
NT1 = 17 * 1024                 # 17 d-planes (plane 16 = zero pad)
PADX = NT1 + 33                 # room for flat-shift overrun
GROUPS = [(0, 0), (1, 0), (0, 1), (1, 1)]   # (sh, sw) partition groups
CLSES = [(0, 0), (0, 1), (1, 0), (1, 1)]    # (ph, pw) output classes
KCLS = [32, 96, 64, 128]        # contiguous K span covering needed groups
CPAIRS = [(0, 2), (1, 3)]       # class pairs, interleaved for PSUM overlap
NDEFER = 6                      # jds whose epilogue defers the bias add


def _khw(p, s):
    # spatial kernel tap index for output parity p / input shift s
    return 1 if p == 0 else (2 if s == 0 else 0)


def _host_tensors(inputs):
    import ml_dtypes
    bf16 = ml_dtypes.bfloat16
    x = np.asarray(inputs["x"], np.float32)
    w = np.asarray(inputs["weight"], np.float32)        # (ci, co, kd, kh, kw)
    gamma = np.asarray(inputs["gamma"], np.float32)
    rvar = np.asarray(inputs["running_var"], np.float32)
    A = gamma / np.sqrt(rvar + EPS)
    wa = w * A[None, :, None, None, None]

    wt = np.zeros((128, 8, 128), np.float32)
    wm = np.zeros((128, 12, 64), np.float32)
    for c, (ph, pw) in enumerate(CLSES):
        for g, (sh, sw) in enumerate(GROUPS):
            if (ph == 0 and sh) or (pw == 0 and sw):
                continue
            kh, kw = _khw(ph, sh), _khw(pw, sw)
            wt[32 * g:32 * g + 32, 2 * c + 0, 0:64] = wa[:, :, 1, kh, kw]
            wt[32 * g:32 * g + 32, 2 * c + 0, 64:128] = wa[:, :, 2, kh, kw]
            wt[32 * g:32 * g + 32, 2 * c + 1, 64:128] = wa[:, :, 0, kh, kw]
            for ti, kd in enumerate((1, 2, 0)):
                wm[32 * g:32 * g + 32, 3 * c + ti, :] = wa[:, :, kd, kh, kw]
    wt = np.ascontiguousarray(wt.reshape(128, 8 * 128)).astype(bf16)
    wm = np.ascontiguousarray(wm.reshape(128, 12 * 64)).astype(bf16)

    xs = []
    for k in range(B):
        xp = np.zeros((CIN, PADX), np.float32)
        xp[:, :D * H * W] = x[k].reshape(CIN, -1)
        xs.append(xp.astype(bf16))
    return xs, wt, wm


def build_nc():
    import concourse.bacc as bacc
    import concourse.mybir as mybir
    import concourse.tile as tile

    f32 = mybir.dt.float32
    bf16 = mybir.dt.bfloat16
    Act = mybir.ActivationFunctionType
    X = mybir.AxisListType.X

    nc = bacc.Bacc()
    x_d = nc.declare_dram_parameter("xp", [CIN, PADX], bf16, isOutput=False)
    wt_d = nc.declare_dram_parameter("wt", [128, 8 * 128], bf16, isOutput=False)
    wm_d = nc.declare_dram_parameter("wm", [128, 12 * 64], bf16, isOutput=False)
    o_d = nc.declare_dram_parameter("out", [COUT, DO, HO, WO], bf16, isOutput=True)

    with tile.TileContext(nc) as tc:
        with (
            tc.tile_pool(name="singles", bufs=1) as sp,
            tc.tile_pool(name="stag", bufs=NDEFER + 4) as stp,
            tc.tile_pool(name="ps", bufs=7, space="PSUM") as pp,
            tc.tile_pool(name="aux", bufs=1, space="PSUM") as ap,
        ):
            # ---------------- loads ----------------
            # Only 5 DMAs are issued up-front (Wt + the 4 chunk-a group
            # loads) -- there are just 8 HWDGE semaphore lanes, and lane
            # reuse makes consumers wait on unrelated later DMAs.  The
            # chunk-b loads and Wm are emitted inside the main loop.
            jtile = sp.tile([128, 512], bf16)
            nc.vector.memset(jtile[:], 0.125)
            Wt = sp.tile([128, 8, 128], bf16)
            Wtf = Wt[:].rearrange("p a b -> p (a b)")
            nc.sync.dma_start(out=Wtf, in_=wt_d[:])
            T1 = sp.tile([128, NT1], bf16)
            CH = 8 * 1024
            for g, (sh, sw) in enumerate(GROUPS):
                off = sh * W + sw
                eng = nc.sync if g < 2 else nc.scalar
                eng.dma_start(out=T1[32 * g:32 * g + 32, 0:CH],
                              in_=x_d[:, off:off + CH])
            Wm = sp.tile([128, 12, 64], bf16)
            T1v = T1[:].rearrange("p (d h w) -> p d h w", d=17, h=H)

            def emit_loads_b():
                for g in range(2):
                    off = GROUPS[g][0] * W + GROUPS[g][1]
                    nc.sync.dma_start(out=T1[32 * g:32 * g + 32, CH:NT1],
                                      in_=x_d[:, off + CH:off + NT1])
                for g in range(2, 4):
                    off = GROUPS[g][0] * W + GROUPS[g][1]
                    nc.gpsimd.dma_start(out=T1[32 * g:32 * g + 32, CH:NT1],
                                        in_=x_d[:, off + CH:off + NT1])

            def emit_load_wm():
                nc.gpsimd.dma_start(out=Wm[:].rearrange("p a b -> p (a b)"),
                                    in_=wm_d[:])

            # junk matmuls: engage the PE clock-gate release ASAP and keep
            # it busy while loads land (dep: just the memset above)
            jps = ap.tile([128, 512], f32, tag="aux")

            def junk(n):
                # upper partition half only: disjoint from the mean
                # accumulator region mps[0:64, 0:1] in the same aux bank
                for _ in range(n):
                    nc.tensor.matmul(jps[64:128, :], jtile[:, 0:64],
                                     jtile[:], start=True, stop=True)

            junk(12)

            # ---------------- mean box sums, directly on T1 ----------------
            # Per group-partition the needed box sums are prefix-range sums
            # of the group's own shifted copy: per-plane full sums plus
            # last-row/col edge sums.  X-reduces are vector-only, so the
            # chunk-b-dependent pieces are emitted mid-loop (below) to keep
            # the vector queue from blocking on the tail of the x load.
            FF = sp.tile([128, 16], f32)
            T1p = T1[:].rearrange("p (d q) -> p d q", d=17)
            nc.vector.reduce_sum(out=FF[:, 0:8], in_=T1p[:, 0:8, :], axis=X)
            cs31 = sp.tile([128, 16], f32)      # last-col sums per plane
            rs31 = sp.tile([128, 16], f32)      # last-row sums per plane
            cor = sp.tile([128, 16], f32)       # corner element per plane
            FE = sp.tile([128, 16], f32)
            EF = sp.tile([128, 16], f32)
            EE = sp.tile([128, 16], f32)
            scolF = sp.tile([128, 12], f32)
            scol = sp.tile([128, 12], bf16)

            def emit_mean_a1():
                nc.vector.reduce_sum(out=FF[:, 8:12], in_=T1p[:, 8:12, :],
                                     axis=X)

            def emit_mean_a2():
                nc.vector.reduce_sum(out=FF[:, 12:16], in_=T1p[:, 12:16, :],
                                     axis=X)
                nc.vector.reduce_sum(out=cs31[:], in_=T1v[:, 0:16, :, 31],
                                     axis=X)
                nc.vector.reduce_sum(out=rs31[:], in_=T1v[:, 0:16, 31, :],
                                     axis=X)
                nc.vector.tensor_copy(out=cor[:], in_=T1v[:, 0:16, 31, 31])

            def emit_mean_b():
                nc.vector.tensor_sub(FE[:], FF[:], cs31[:])
                nc.vector.tensor_sub(EF[:], FF[:], rs31[:])
                nc.vector.tensor_sub(EE[:], FE[:], rs31[:])
                nc.vector.tensor_add(EE[:], EE[:], cor[:])
                combos = {(0, 0): FF, (0, 1): FE, (1, 0): EF, (1, 1): EE}
                dr = [(0, 16), (0, 15), (1, 16)]    # d range per (pd,sd) tap
                for c, (ph, pw) in enumerate(CLSES):
                    cmb = combos[(ph, pw)]
                    for ti, (a, b) in enumerate(dr):
                        nc.vector.reduce_sum(
                            out=scolF[:, 3 * c + ti:3 * c + ti + 1],
                            in_=cmb[:, a:b], axis=X)
                nc.vector.tensor_copy(out=scol[:], in_=scolF[:])

            mps = ap.tile([128, 512], f32, tag="aux")
            bcol = sp.tile([64, 1], f32)
            brep = sp.tile([128, 1], f32)

            # ---------------- main loop ----------------
            epi = 0
            oq = 0
            out_engs = [nc.sync, nc.scalar, nc.gpsimd]
            stags = []

            def emit_mms(jd, nt, last, nfill=(0, 0)):
                # two class-pairs, passes interleaved so same-bank
                # accumulate pairs never run back-to-back on the PE
                pss = [None] * 4
                for pr, (ca, cb) in enumerate(CPAIRS):
                    for c in (ca, cb):
                        pss[c] = pp.tile([128, 512], f32, tag="ps",
                                         name=f"ps{c}")
                    for pi in range(1 if last else 2):
                        for c in (ca, cb):
                            K = KCLS[c]
                            nc.tensor.matmul(
                                pss[c][:, :], Wt[0:K, 2 * c + pi, :],
                                T1v[0:K, jd + pi, 16 * nt:16 * nt + 16, :],
                                start=(pi == 0), stop=(pi == 1 or last))
                    junk(nfill[pr])
                return pss

            def emit_epi(stag, ps, c, nt, defer, last):
                nonlocal epi
                ph, pw = CLSES[c]
                np_ = 64 if last else 128
                jhc = 16 if (ph == 0 or nt == 0) else 15
                jwc = W - pw
                h0_ = 32 * nt + ph
                psv = ps[:].rearrange("p (a b) -> p a b", a=16)
                dest = stag[0:np_, h0_:min(h0_ + 2 * jhc, HO):2,
                            pw:min(pw + 2 * jwc, WO):2]
                src = psv[0:np_, 0:jhc, 0:jwc]
                if defer:
                    if epi % 2 == 0:
                        nc.scalar.activation(out=dest, in_=src, func=Act.Copy,
                                             bias=0.0, scale=1.0)
                    else:
                        nc.vector.tensor_copy(out=dest, in_=src)
                elif epi % 2 == 0:
                    nc.scalar.activation(out=dest, in_=src, func=Act.Identity,
                                         bias=brep[0:np_], scale=1.0)
                else:
                    nc.vector.tensor_scalar_add(dest, src, brep[0:np_])
                epi += 1

            def emit_out(stag, jd, n_planes=2):
                nonlocal oq
                for q in range(n_planes):
                    eng = out_engs[oq % 3]
                    oq += 1
                    eng.dma_start(out=o_d[:, 2 * jd + q, :, :],
                                  in_=stag[64 * q:64 * q + 64, :, :])

            # jd -> deferred index to fix there (spread over the tail)
            fix_at = {6: 0, 8: 1, 10: 2, 12: 3, 14: 4, 15: 5}

            for jd in range(16):
                last = jd == 15
                defer = jd < NDEFER
                stag = stp.tile([128, HO, WO], bf16)
                for nt in range(2):
                    nfill = (0, 0)
                    if jd == 0:
                        nfill = (14, 3) if nt == 0 else (3, 3)
                    elif jd == 1:
                        nfill = (3, 2)
                    elif jd == 2:
                        nfill = (2, 2)
                    elif jd == 3:
                        nfill = (1, 1)
                    elif jd in fix_at:
                        nfill = (2, 2)
                    pss = emit_mms(jd, nt, last, nfill)
                    if jd == NDEFER - 1 and nt == 1:
                        # junk covers the wait for scol and keeps the PE
                        # activity monitor from re-throttling through the
                        # tiny-N mean matmuls; bcol's RAW producer must
                        # stay the last mean matmul
                        junk(6)
                        for pi in range(12):
                            nc.tensor.matmul(mps[0:64, 0:1], Wm[:, pi, :],
                                             scol[:, pi:pi + 1],
                                             start=(pi == 0), stop=(pi == 11))
                            if pi < 11:
                                junk(1)
                    for c in range(4):
                        emit_epi(stag, pss[c], c, nt, defer, last)
                if defer:
                    stags.append(stag)
                else:
                    emit_out(stag, jd, 1 if last else 2)
                if jd == 1:
                    emit_loads_b()
                    emit_load_wm()
                elif jd == 4:
                    emit_mean_a1()
                    emit_mean_a2()
                elif jd == NDEFER - 1:
                    emit_mean_b()
                    nc.scalar.activation(out=bcol[:], in_=mps[0:64, 0:1],
                                         func=Act.Copy, bias=0.0,
                                         scale=-1.0 / NSPAT)
                    nc.vector.tensor_copy(out=brep[0:64], in_=bcol[:])
                    nc.gpsimd.dma_start(out=brep[64:128], in_=bcol[:])
                if jd in fix_at:
                    # deferred-bias fix-up on the scalar engine (vector's
                    # in-place add and all gpsimd tensor ops are slow),
                    # split in half to soften the epilogue-drain stall
                    i = fix_at[jd]
                    stf = stags[i][:].rearrange("p h w -> p (h w)")
                    hwm = (HO * WO) // 2
                    nc.scalar.activation(out=stf[:, 0:hwm], in_=stf[:, 0:hwm],
                                         func=Act.Identity,
                                         bias=brep[:], scale=1.0)
                    nc.scalar.activation(out=stf[:, hwm:], in_=stf[:, hwm:],
                                         func=Act.Identity,
                                         bias=brep[:], scale=1.0)
                    emit_out(stags[i], i, 2)
    nc.compile()
    return nc


_NC = None


def _get_nc():
    global _NC
    if _NC is None:
        _NC = build_nc()
    return _NC


def _in_maps(inputs):
    xs, wt, wm = _host_tensors(inputs)
    return [{"xp": xs[k], "wt": wt, "wm": wm} for k in range(B)]


def run(inputs, trace=False):
    from concourse.bass_utils import run_bass_kernel_spmd

    nc = _get_nc()
    res = run_bass_kernel_spmd(nc, _in_maps(inputs),
                               core_ids=list(range(B)), trace=trace)
    out = np.stack([np.asarray(res.results[k]["out"], np.float32)
                    for k in range(B)], axis=0)
    return out, res


def kernel(**inputs) -> np.ndarray:
    out, _ = run(inputs, trace=False)
    return out


# ---------------------------------------------------------------------------
# Benchmarking helpers (test.py only; the grader uses kernel() above).
# ---------------------------------------------------------------------------

def enable_axon_profiling():
    """Register the missing antenv.axon_hooks shim so that
    run_bass_kernel_spmd(trace=True) can capture NTFF profiles through the
    axon PJRT .so (see trn_agent_boot.trn_boot)."""
    import sys
    import types
    try:
        import antenv.axon_hooks  # noqa: F401
        return True
    except ImportError:
        pass
    mod = types.ModuleType("antenv.axon_hooks")
    mod._hook = None

    def set_axon_ntff_profile_hook(h):
        mod._hook = h

    def get_axon_ntff_profile_hook():
        return mod._hook

    mod.set_axon_ntff_profile_hook = set_axon_ntff_profile_hook
    mod.get_axon_ntff_profile_hook = get_axon_ntff_profile_hook
    sys.modules["antenv.axon_hooks"] = mod
    import antenv
    antenv.axon_hooks = mod
    from trn_agent_boot.trn_boot import _ntff_profile_via_ctypes
    hook = _ntff_profile_via_ctypes('/opt/axon/libaxon_pjrt.so')
    if hook is None:
        return False
    mod._hook = hook
    return True


# revision 27
# speedup vs baseline: 1.0566x; 1.0566x over previous
"""Trainium2 Bass kernel for ConvTranspose3d(32->64, k=3, s=2, p=1) + inference
BatchNorm + per-(sample,channel) spatial mean subtraction.

Math: bias / beta / running_mean cancel exactly in the mean subtraction:
    out = A_c * (convT(x) - mean_spatial(convT(x))),  A_c = gamma/sqrt(var+eps)
A_c is folded into the weights on the host, so the device epilogue is a
single per-channel bias add.

Decomposition: stride-2 transpose conv -> 8 output parity classes.
x arrives pre-cast to bf16; the 4 (sh,sw) spatial shift variants live in 4
partition groups of one SBUF tensor T1 (128 = 4x32ci partitions), loaded
directly from a host-padded flat copy of x at 4 byte offsets (pad-free
32x32 plane layout; shift overruns read neighbouring-plane garbage that
only feeds trimmed grid positions).  Per (ph,pw) class both d-parities are
computed by 2 matmuls (M=128 = pd x co; pass 1 reads plane jd with kd=1|2
weights, pass 2 reads plane jd+1 with kd=0 weights in the upper half).

The spatial mean: per shift-group the needed 12 box sums of x equal plain
prefix-range sums of that group's own shifted copy (shifted data + fixed
local range == fixed data + shifted range), so they are computed directly
on T1's 128 partitions from per-plane full sums + last-row/col edge sums,
then fed through 12 tiny matmuls into the epilogue bias.  Epilogues for
jd0-4 run bias-free (plain copy) and are fixed up in stag afterwards, so
the PE never stalls on the mean during the load ramp.

Sharding: data-parallel, one sample per core (B=8, 8 cores).
"""

import numpy as np

B, CIN, COUT = 8, 32, 64
D, H, W = 16, 32, 32
DO, HO, WO = 31, 63, 63
EPS = 1e-5
NSPAT = DO * HO * WO
NT1 = 17 * 1024                 # 17 d-planes (plane 16 = zero pad)
PADX = NT1 + 33                 # room for flat-shift overrun
GROUPS = [(0, 0), (1, 0), (0, 1), (1, 1)]   # (sh, sw) partition groups
CLSES = [(0, 0), (0, 1), (1, 0), (1, 1)]    # (ph, pw) output classes
KCLS = [32, 96, 64, 128]        # contiguous K span covering needed groups
CPAIRS = [(0, 2), (1, 3)]       # class pairs, interleaved for PSUM overlap
NDEFER = 6                      # jds whose epilogue defers the bias add


def _khw(p, s):
    # spatial kernel tap index for output parity p / input shift s
    return 1 if p == 0 else (2 if s == 0 else 0)


def _host_tensors(inputs):
    import ml_dtypes
    bf16 = ml_dtypes.bfloat16
    x = np.asarray(inputs["x"], np.float32)
    w = np.asarray(inputs["weight"], np.float32)        # (ci, co, kd, kh, kw)
    gamma = np.asarray(inputs["gamma"], np.float32)
    rvar = np.asarray(inputs["running_var"], np.float32)
    A = gamma / np.sqrt(rvar + EPS)
    wa = w * A[None, :, None, None, None]

    wt = np.zeros((128, 8, 128), np.float32)
    wm = np.zeros((128, 12, 64), np.float32)
    for c, (ph, pw) in enumerate(CLSES):
        for g, (sh, sw) in enumerate(GROUPS):
            if (ph == 0 and sh) or (pw == 0 and sw):
                continue
            kh, kw = _khw(ph, sh), _khw(pw, sw)
            wt[32 * g:32 * g + 32, 2 * c + 0, 0:64] = wa[:, :, 1, kh, kw]
            wt[32 * g:32 * g + 32, 2 * c + 0, 64:128] = wa[:, :, 2, kh, kw]
            wt[32 * g:32 * g + 32, 2 * c + 1, 64:128] = wa[:, :, 0, kh, kw]
            for ti, kd in enumerate((1, 2, 0)):
                wm[32 * g:32 * g + 32, 3 * c + ti, :] = wa[:, :, kd, kh, kw]
    wt = np.ascontiguousarray(wt.reshape(128, 8 * 128)).astype(bf16)
    wm = np.ascontiguousarray(wm.reshape(128, 12 * 64)).astype(bf16)

    xs = []
    for k in range(B):
        xp = np.zeros((CIN, PADX), np.float32)
        xp[:, :D * H * W] = x[k].reshape(CIN, -1)
        xs.append(xp.astype(bf16))
    return xs, wt, wm


def build_nc():
    import concourse.bacc as bacc
    import concourse.mybir as mybir
    import concourse.tile as tile

    f32 = mybir.dt.float32
    bf16 = mybir.dt.bfloat16
    Act = mybir.ActivationFunctionType
    X = mybir.AxisListType.X

    nc = bacc.Bacc()
    x_d = nc.declare_dram_parameter("xp", [CIN, PADX], bf16, isOutput=False)
    wt_d = nc.declare_dram_parameter("wt", [128, 8 * 128], bf16, isOutput=False)
    wm_d = nc.declare_dram_parameter("wm", [128, 12 * 64], bf16, isOutput=False)
    o_d = nc.declare_dram_parameter("out", [COUT, DO, HO, WO], bf16, isOutput=True)

    with tile.TileContext(nc) as tc:
        with (
            tc.tile_pool(name="singles", bufs=1) as sp,
            tc.tile_pool(name="stag", bufs=NDEFER + 4) as stp,
            tc.tile_pool(name="ps", bufs=3, space="PSUM") as pp,
            tc.tile_pool(name="aux", bufs=1, space="PSUM") as ap,
        ):
            # ---------------- loads ----------------
            # Only 5 DMAs are issued up-front (Wt + the 4 chunk-a group
            # loads) -- there are just 8 HWDGE semaphore lanes, and lane
            # reuse makes consumers wait on unrelated later DMAs.  The
            # chunk-b loads and Wm are emitted inside the main loop.
            jtile = sp.tile([128, 512], bf16)
            nc.vector.memset(jtile[:], 0.125)
            Wt = sp.tile([128, 8, 128], bf16)
            Wtf = Wt[:].rearrange("p a b -> p (a b)")
            nc.sync.dma_start(out=Wtf, in_=wt_d[:])
            T1 = sp.tile([128, NT1], bf16)
            CH = 8 * 1024
            for g, (sh, sw) in enumerate(GROUPS):
                off = sh * W + sw
                eng = nc.sync if g < 2 else nc.scalar
                eng.dma_start(out=T1[32 * g:32 * g + 32, 0:CH],
                              in_=x_d[:, off:off + CH])
            Wm = sp.tile([128, 12, 64], bf16)
            T1v = T1[:].rearrange("p (d h w) -> p d h w", d=17, h=H)

            def emit_loads_b():
                for g in range(2):
                    off = GROUPS[g][0] * W + GROUPS[g][1]
                    nc.sync.dma_start(out=T1[32 * g:32 * g + 32, CH:NT1],
                                      in_=x_d[:, off + CH:off + NT1])
                for g in range(2, 4):
                    off = GROUPS[g][0] * W + GROUPS[g][1]
                    nc.gpsimd.dma_start(out=T1[32 * g:32 * g + 32, CH:NT1],
                                        in_=x_d[:, off + CH:off + NT1])

            def emit_load_wm():
                nc.gpsimd.dma_start(out=Wm[:].rearrange("p a b -> p (a b)"),
                                    in_=wm_d[:])

            # junk matmuls: engage the PE clock-gate release ASAP and keep
            # it busy while loads land (dep: just the memset above)
            jps = ap.tile([128, 512], f32, tag="aux")

            def junk(n):
                # upper partition half only: disjoint from the mean
                # accumulator region mps[0:64, 0:1] in the same aux bank
                for _ in range(n):
                    nc.tensor.matmul(jps[64:128, :], jtile[:, 0:64],
                                     jtile[:], start=True, stop=True)

            junk(12)

            # ---------------- mean box sums, directly on T1 ----------------
            # Per group-partition the needed box sums are prefix-range sums
            # of the group's own shifted copy: per-plane full sums plus
            # last-row/col edge sums.  X-reduces are vector-only, so the
            # chunk-b-dependent pieces are emitted mid-loop (below) to keep
            # the vector queue from blocking on the tail of the x load.
            FF = sp.tile([128, 16], f32)
            T1p = T1[:].rearrange("p (d q) -> p d q", d=17)
            nc.vector.reduce_sum(out=FF[:, 0:8], in_=T1p[:, 0:8, :], axis=X)
            cs31 = sp.tile([128, 16], f32)      # last-col sums per plane
            rs31 = sp.tile([128, 16], f32)      # last-row sums per plane
            cor = sp.tile([128, 16], f32)       # corner element per plane
            FE = sp.tile([128, 16], f32)
            EF = sp.tile([128, 16], f32)
            EE = sp.tile([128, 16], f32)
            scolF = sp.tile([128, 12], f32)
            scol = sp.tile([128, 12], bf16)

            def emit_mean_a1():
                nc.vector.reduce_sum(out=FF[:, 8:12], in_=T1p[:, 8:12, :],
                                     axis=X)

            def emit_mean_a2():
                nc.vector.reduce_sum(out=FF[:, 12:16], in_=T1p[:, 12:16, :],
                                     axis=X)
                nc.vector.reduce_sum(out=cs31[:], in_=T1v[:, 0:16, :, 31],
                                     axis=X)
                nc.vector.reduce_sum(out=rs31[:], in_=T1v[:, 0:16, 31, :],
                                     axis=X)
                nc.vector.tensor_copy(out=cor[:], in_=T1v[:, 0:16, 31, 31])

            def emit_mean_b():
                nc.vector.tensor_sub(FE[:], FF[:], cs31[:])
                nc.vector.tensor_sub(EF[:], FF[:], rs31[:])
                nc.vector.tensor_sub(EE[:], FE[:], rs31[:])
                nc.vector.tensor_add(EE[:], EE[:], cor[:])
                combos = {(0, 0): FF, (0, 1): FE, (1, 0): EF, (1, 1): EE}
                dr = [(0, 16), (0, 15), (1, 16)]    # d range per (pd,sd) tap
                for c, (ph, pw) in enumerate(CLSES):
                    cmb = combos[(ph, pw)]
                    for ti, (a, b) in enumerate(dr):
                        nc.vector.reduce_sum(
                            out=scolF[:, 3 * c + ti:3 * c + ti + 1],
                            in_=cmb[:, a:b], axis=X)
                nc.vector.tensor_copy(out=scol[:], in_=scolF[:])

            mps = ap.tile([128, 512], f32, tag="aux")
            bcol = sp.tile([64, 1], f32)
            brep = sp.tile([128, 1], f32)

            # ---------------- main loop ----------------
            epi = 0
            oq = 0
            out_engs = [nc.sync, nc.scalar, nc.gpsimd]
            stags = []

            def emit_pair(jd, pr, last, nfill=0):
                # one class-pair; both nt halves go into a 2-bank psum
                # tile, passes interleaved so same-bank accumulate pairs
                # never run back-to-back on the PE
                ca, cb = CPAIRS[pr]
                ps2 = {}
                for c in (ca, cb):
                    ps2[c] = pp.tile([128, 1024], f32, tag="ps",
                                     name=f"ps{c}")
                for nt in range(2):
                    for pi in range(1 if last else 2):
                        for c in (ca, cb):
                            K = KCLS[c]
                            nc.tensor.matmul(
                                ps2[c][:, 512 * nt:512 * nt + 512],
                                Wt[0:K, 2 * c + pi, :],
                                T1v[0:K, jd + pi, 16 * nt:16 * nt + 16, :],
                                start=(pi == 0), stop=(pi == 1 or last))
                junk(nfill)
                return ps2

            def emit_epi(stag, ps, c, defer, last):
                nonlocal epi
                ph, pw = CLSES[c]
                np_ = 64 if last else 128
                jhc = 32 if ph == 0 else 31
                jwc = W - pw
                psv = ps[:].rearrange("p (a b) -> p a b", a=32)
                dest = stag[0:np_, ph:min(ph + 2 * jhc, HO):2,
                            pw:min(pw + 2 * jwc, WO):2]
                src = psv[0:np_, 0:jhc, 0:jwc]
                if defer:
                    if epi % 2 == 0:
                        nc.scalar.activation(out=dest, in_=src, func=Act.Copy,
                                             bias=0.0, scale=1.0)
                    else:
                        nc.vector.tensor_copy(out=dest, in_=src)
                elif epi % 2 == 0:
                    nc.scalar.activation(out=dest, in_=src, func=Act.Identity,
                                         bias=brep[0:np_], scale=1.0)
                else:
                    nc.vector.tensor_scalar_add(dest, src, brep[0:np_])
                epi += 1

            def emit_out(stag, jd, n_planes=2):
                nonlocal oq
                for q in range(n_planes):
                    eng = out_engs[oq % 3]
                    oq += 1
                    eng.dma_start(out=o_d[:, 2 * jd + q, :, :],
                                  in_=stag[64 * q:64 * q + 64, :, :])

            # jd -> deferred index to fix there (all done by jd14 so the
            # stag ring never catches its own pending fix-up)
            fix_at = {6: 0, 8: 1, 10: 2, 12: 3, 13: 4, 14: 5}

            # ---- phase A: classes (0,2) for jd0..5 -- need only g0/g1,
            # so the PE does real work while g2/g3 stream in
            for jd in range(NDEFER):
                stag = stp.tile([128, HO, WO], bf16, tag="st", name=f"stagA{jd}")
                stags.append(stag)
                p = emit_pair(jd, 0, False, nfill=2 if jd == 0 else 0)
                for c in CPAIRS[0]:
                    emit_epi(stag, p[c], c, True, False)
                if jd == 1:
                    emit_loads_b()
                    emit_load_wm()

            # ---- phase B: classes (1,3) for jd0..5
            for jd in range(NDEFER):
                p = emit_pair(jd, 1, False, nfill=1 if jd == 0 else 0)
                for c in CPAIRS[1]:
                    emit_epi(stags[jd], p[c], c, True, False)
                if jd == 2:
                    emit_mean_a1()
                elif jd == 3:
                    emit_mean_a2()
                elif jd == 4:
                    emit_mean_b()

            # ---- mean matmuls + bias (junk keeps the activity monitor
            # from re-throttling through the tiny-N matmuls)
            junk(4)
            for pi in range(12):
                nc.tensor.matmul(mps[0:64, 0:1], Wm[:, pi, :],
                                 scol[:, pi:pi + 1],
                                 start=(pi == 0), stop=(pi == 11))
                if pi < 11:
                    junk(1)
            nc.scalar.activation(out=bcol[:], in_=mps[0:64, 0:1],
                                 func=Act.Copy, bias=0.0,
                                 scale=-1.0 / NSPAT)
            nc.vector.tensor_copy(out=brep[0:64], in_=bcol[:])
            nc.gpsimd.dma_start(out=brep[64:128], in_=bcol[:])

            # ---- normal jds 6..15
            for jd in range(NDEFER, 16):
                last = jd == 15
                stag = stp.tile([128, HO, WO], bf16, tag="st", name=f"stagN{jd}")
                nf = 1 if jd in fix_at else 0
                pA = emit_pair(jd, 0, last, nfill=nf)
                pB = emit_pair(jd, 1, last, nfill=nf)
                for c in (0, 2, 1, 3):
                    emit_epi(stag, (pA if c in CPAIRS[0] else pB)[c], c,
                             False, last)
                emit_out(stag, jd, 1 if last else 2)
                if jd in fix_at:
                    # deferred-bias fix-up on the scalar engine, split in
                    # half to soften the epilogue-drain stall
                    i = fix_at[jd]
                    stf = stags[i][:].rearrange("p h w -> p (h w)")
                    hwm = (HO * WO) // 2
                    nc.scalar.activation(out=stf[:, 0:hwm], in_=stf[:, 0:hwm],
                                         func=Act.Identity,
                                         bias=brep[:], scale=1.0)
                    nc.scalar.activation(out=stf[:, hwm:], in_=stf[:, hwm:],
                                         func=Act.Identity,
                                         bias=brep[:], scale=1.0)
                    emit_out(stags[i], i, 2)
    nc.compile()
    return nc


_NC = None


def _get_nc():
    global _NC
    if _NC is None:
        _NC = build_nc()
    return _NC


def _in_maps(inputs):
    xs, wt, wm = _host_tensors(inputs)
    return [{"xp": xs[k], "wt": wt, "wm": wm} for k in range(B)]


def run(inputs, trace=False):
    from concourse.bass_utils import run_bass_kernel_spmd

    nc = _get_nc()
    res = run_bass_kernel_spmd(nc, _in_maps(inputs),
                               core_ids=list(range(B)), trace=trace)
    out = np.stack([np.asarray(res.results[k]["out"], np.float32)
                    for k in range(B)], axis=0)
    return out, res


def kernel(**inputs) -> np.ndarray:
    out, _ = run(inputs, trace=False)
    return out


# ---------------------------------------------------------------------------
# Benchmarking helpers (test.py only; the grader uses kernel() above).
# ---------------------------------------------------------------------------

def enable_axon_profiling():
    """Register the missing antenv.axon_hooks shim so that
    run_bass_kernel_spmd(trace=True) can capture NTFF profiles through the
    axon PJRT .so (see trn_agent_boot.trn_boot)."""
    import sys
    import types
    try:
        import antenv.axon_hooks  # noqa: F401
        return True
    except ImportError:
        pass
    mod = types.ModuleType("antenv.axon_hooks")
    mod._hook = None

    def set_axon_ntff_profile_hook(h):
        mod._hook = h

    def get_axon_ntff_profile_hook():
        return mod._hook

    mod.set_axon_ntff_profile_hook = set_axon_ntff_profile_hook
    mod.get_axon_ntff_profile_hook = get_axon_ntff_profile_hook
    sys.modules["antenv.axon_hooks"] = mod
    import antenv
    antenv.axon_hooks = mod
    from trn_agent_boot.trn_boot import _ntff_profile_via_ctypes
    hook = _ntff_profile_via_ctypes('/opt/axon/libaxon_pjrt.so')
    if hook is None:
        return False
    mod._hook = hook
    return True
